# revision 31
# baseline (speedup 1.0000x reference)
"""Tensor-parallel MultiHeadAttention (QKV + RoPE + GQA causal SDPA + dense)
for 8 Trainium2 NeuronCores — bf16, software-pipelined edition.

Sharding (TP as in TPMultiHeadAttention): core d owns query heads {2d, 2d+1}
and the single kv head d//2 (kv heads replicated across core pairs), plus the
matching 256 columns of the dense projection. Each core produces a full-shape
partial output; the all-reduce is a host-side sum over the 8 bf16 partials.

All matmul operands are bf16 (same PE column rate as float32r but half the
LDWEIGHTS time, half the DMA bytes, 2x DVE rate); PSUM accumulation is fp32.

Schedule highlights (from perfetto analysis of earlier revisions):
  - exp on ScalarE (616ns/tile) is slower than a score+ctx matmul pair
    (432ns), so the two heads' score tiles share one 2-bank PSUM tile and a
    single exp instruction covers both ([128, 2, n]).
  - attention for chunks 0 and 1 trickles INSIDE the QKV phase (the PE has
    54us of projection work to hide the exp latency); chunks 2 and 3 are
    software-pipelined with ctx lagging scores, with dense units of earlier
    chunks metered in as PE filler.
  - v^T -> v transposes use the DMA XBAR (16-bit only), freeing PSUM banks
    and the PE; rotate_half is two partition-shifted DVE copies.
  - softmax denominators: bf16 DVE accumulation chains, column-summed by a
    ones-vector matmul (borrowing a PSUM bank from the sd/o rings), DVE
    reciprocal, gpsimd partition broadcast.  The ctx PSUM banks are released
    early by an unscaled ScalarE copy; the 1/denominator multiply happens in
    SBUF off the ring-critical path.  gpsimd's ucode library load (~7us) is
    prefetched by a dummy broadcast at kernel start, and the rope/mask
    tables ride the otherwise-idle gpsimd DMA ring.
"""

import numpy as np
import ml_dtypes

B, S, E = 1, 2048, 2048
H, KVH, D = 16, 4, 128
NCORES = 8
P = 128
FD = 512            # matmul moving free dim == one fp32 PSUM bank
NE = E // P         # 16 contraction tiles over the embedding dim
NG = 4              # eo-groups of 4 (one 512KB DMA each)
NSC = S // FD       # 4 sequence chunks
NST = S // P        # 16 sequence tiles
FLOC = 4 * P        # local fused qkv rows per core (2 q heads + k + v)
ROPE_BASE = 10000.0
# causally visible query sub-range start for diagonal sk tile o
DIAG_START = (0, 128, 256, 384)
BF = ml_dtypes.bfloat16

LAST_RESULT = None
_BASS_CACHE = None


def _rope_tables():
    inv = 1.0 / (ROPE_BASE ** (np.arange(0, D, 2, dtype=np.float64) / D))
    t = np.arange(S, dtype=np.float64)
    freqs = np.outer(t, inv)
    emb = np.concatenate([freqs, freqs], axis=-1)  # [S, D]
    return np.cos(emb), np.sin(emb)


def _host_constants():
    cos, sin = _rope_tables()
    cos_ds = np.ascontiguousarray(cos.T)  # [D, S]
    sin_ds = np.ascontiguousarray(sin.T)
    # sign-folded sin for the partition-shifted rotate-half:
    # tt[d] = qt[(d+64)%128] * sg[d],  sg = [-sin[:64]; +sin[64:]]
    sg = np.concatenate([-sin_ds[:64], sin_ds[64:]], axis=0)
    r_idx = np.arange(P)[:, None]
    c_idx = np.arange(P)[None, :]
    tri = (r_idx <= c_idx).astype(np.float64)
    return {
        "cosr": cos_ds.astype(BF),
        "sgsin": sg.astype(BF),
        "trim": tri.astype(BF),
        "ones": np.ones((P, 1), np.float64).astype(BF),
    }


def _build_bass():
    import concourse.mybir as mybir
    import concourse.tile as tile
    from concourse import bacc

    f32 = mybir.dt.float32
    bf16 = mybir.dt.bfloat16
    Exp = mybir.ActivationFunctionType.Exp

    nc = bacc.Bacc(None, target_bir_lowering=False, name="mha_tp8_v3")
    xG = nc.dram_tensor("xG", [NSC, NG, P, 4, FD], bf16, kind="ExternalInput")
    wG = nc.dram_tensor("wG", [NG, P, 4, FLOC], bf16, kind="ExternalInput")
    wdG = nc.dram_tensor("wdG", [P, 2, S], bf16, kind="ExternalInput")
    cosr = nc.dram_tensor("cosr", [P, S], bf16, kind="ExternalInput")
    sgsin = nc.dram_tensor("sgsin", [P, S], bf16, kind="ExternalInput")
    trim = nc.dram_tensor("trim", [P, P], bf16, kind="ExternalInput")
    ones = nc.dram_tensor("ones", [P, 1], bf16, kind="ExternalInput")
    out = nc.dram_tensor("out", [NSC, 4, P, 4, FD], bf16, kind="ExternalOutput")

    with tile.TileContext(nc) as tc:
        with tc.tile_pool(name="const", bufs=1) as const, \
             tc.tile_pool(name="ps_ctx", bufs=2, space="PSUM") as ps_ctx, \
             tc.tile_pool(name="xs_p", bufs=6) as xpool, \
             tc.tile_pool(name="rtmp", bufs=3) as rtmp, \
             tc.tile_pool(name="pt_p", bufs=8) as ptp, \
             tc.tile_pool(name="acc_p", bufs=2) as accp, \
             tc.tile_pool(name="dn_p", bufs=2) as dnp, \
             tc.tile_pool(name="ctx_p", bufs=3) as ctxp, \
             tc.tile_pool(name="out_p", bufs=3) as outp:
            w_sb = const.tile([P, NE, FLOC], bf16, name="w_sb")
            cq = const.tile([P, S], bf16, name="cq")
            sg = const.tile([P, S], bf16, name="sg")
            mk = const.tile([P, P], bf16, name="mk")
            wd_sb = const.tile([P, 2, S], bf16, name="wd_sb")
            qr = const.tile([P, 2, S], bf16, name="qr")
            kr = const.tile([P, S], bf16, name="kr")
            vT = const.tile([P, S], bf16, name="vT")
            vn = const.tile([P, NST, P], bf16, name="vn")
            on = const.tile([P, 1], bf16, name="on")
            warm = const.tile([P, 8], bf16, name="warm")

            # ---- shared attention machinery (paired heads per j-tile) ----
            st_ = {}          # per-chunk attention state
            all_csb = {}

            def attn_begin(c):
                two = c >= 1
                ctxps = [
                    ps_ctx.tile([P, FD], f32, tag="ctx", name=f"ctx_{c}_{h}")
                    for h in range(2)
                ]
                accs = [
                    accp.tile([P, 2, FD], bf16, tag=f"acc{ch}", name=f"acc_{c}_{ch}")
                    for ch in range(2 if two else 1)
                ]
                st_[c] = (ctxps, accs, two)

            def attn_step(c, j, sdpool):
                ctxps, accs, two = st_[c]
                o = j - 4 * c
                so = DIAG_START[o] if o >= 0 else 0
                n = FD - so
                sd = sdpool.tile([P, 2, FD], f32, tag="sd", name=f"sd_{c}_{j}")
                for h in range(2):
                    nc.tensor.matmul(
                        sd[:, h, :n],
                        lhsT=kr[:, j * P:(j + 1) * P],
                        rhs=qr[:, h, c * FD + so:(c + 1) * FD],
                        start=True, stop=True,
                    )
                pt = ptp.tile([P, 2, FD], bf16, tag="pt", name=f"pt_{c}_{j}")
                nc.scalar.activation(pt[:, :, :n], sd[:, :, :n], Exp)
                if o >= 0:
                    for h in range(2):
                        nc.vector.tensor_mul(pt[:, h, :P], pt[:, h, :P], mk)
                acc = accs[j % 2 if two else 0]
                if j < (2 if two else 1):
                    nc.vector.tensor_copy(acc, pt)
                else:
                    nc.vector.tensor_add(acc[:, :, so:], acc[:, :, so:], pt[:, :, :n])
                return (j, pt, so, n)

            def attn_ctx(c, ent):
                ctxps, _, _ = st_[c]
                nj = 4 * c + 4
                j, pt, so, n = ent
                for h in range(2):
                    nc.tensor.matmul(
                        ctxps[h][:, so:],
                        lhsT=vn[:, j, :],
                        rhs=pt[:, h, :n],
                        start=(j == 0), stop=(j == nj - 1),
                    )

            def attn_tail(c, sdpool, ptag="sd"):
                ctxps, accs, two = st_[c]
                crs = []
                for h in range(2):
                    # unscaled PSUM->SBUF copy releases the ctx bank ~0.7us
                    # after the last ctx matmul (the scale chain is ~2.5us
                    # and would otherwise stall the next chunk's ctx ring)
                    cr = ctxp.tile([P, FD], bf16, tag=f"cr{h}", name=f"cr_{c}_{h}")
                    nc.scalar.copy(cr, ctxps[h])
                    crs.append(cr)
                for h in range(2):
                    # column sums via a ones-vector matmul into the sd ring
                    rp = sdpool.tile([1, FD], f32, tag=ptag, name=f"rp_{c}_{h}")
                    nc.tensor.matmul(rp, lhsT=on, rhs=accs[0][:, h, :],
                                     start=True, stop=not two)
                    if two:
                        nc.tensor.matmul(rp, lhsT=on, rhs=accs[1][:, h, :],
                                         start=False, stop=True)
                    rec = dnp.tile([1, FD], f32, tag=f"rec{h}", name=f"rec_{c}_{h}")
                    nc.vector.reciprocal_approx_fast(rec, rp)
                    rb = dnp.tile([P, FD], f32, tag=f"rb{h}", name=f"rb_{c}_{h}")
                    nc.gpsimd.partition_broadcast(rb, rec)
                    ct = ctxp.tile([P, FD], bf16, tag=f"ct{h}", name=f"csb_{c}_{h}")
                    nc.vector.tensor_mul(ct, crs[h], rb)
                    all_csb[(c, h)] = ct

            # ---- Phase A: QKV + RoPE + v-transpose, attn(0/1) trickled ----
            with tc.tile_pool(name="ps_qkv", bufs=1, space="PSUM") as pqkv, \
                 tc.tile_pool(name="ps_sA", bufs=1, space="PSUM") as psA:
                # mask + ucode-library prefetch on the gpsimd ring; the 1MB
                # cos/sin tables issue on the scalar HWDGE ring AFTER the
                # chunk-0 weight issues (ring order defers them out of the
                # startup-critical window; SWDGE is too slow when late)
                nc.gpsimd.dma_start(mk, trim[:, :])
                nc.gpsimd.partition_broadcast(warm, mk[0:1, 0:8])
                nc.gpsimd.dma_start(on, ones[:, :])

                qkv_ps = {}

                def qkv_units(sc):
                    """Yields after each (g, j) group of 4 matmuls (~864ns PE)."""
                    psums = [
                        pqkv.tile([P, FD], f32, tag=f"qkv{f}", name=f"ps_qkv{f}_{sc}")
                        for f in range(4)
                    ]
                    qkv_ps[sc] = psums
                    for g in range(NG):
                        fine = sc == 0 and g == 0
                        if sc == 0 and not fine:
                            nc.scalar.dma_start(w_sb[:, 4 * g:4 * g + 4, :], wG[g])
                        xs = xpool.tile([P, 4, FD], bf16, tag="xs", name=f"xs_{sc}_{g}")
                        if not fine:
                            nc.sync.dma_start(xs, xG[sc, g])
                        for j in range(4):
                            if fine:
                                # 128KB pieces, weights on the idle scalar
                                # ring, so the first matmuls aren't starved
                                nc.scalar.dma_start(w_sb[:, j, :], wG[0, :, j, :])
                                nc.sync.dma_start(xs[:, j, :], xG[0, 0, :, j, :])
                            eo = 4 * g + j
                            for f in range(4):
                                nc.tensor.matmul(
                                    psums[f],
                                    lhsT=w_sb[:, eo, f * P:(f + 1) * P],
                                    rhs=xs[:, j, :],
                                    start=(eo == 0),
                                    stop=(eo == NE - 1),
                                )
                            yield
                    if sc == 0:
                        nc.scalar.dma_start(cq, cosr[:, :])
                        nc.scalar.dma_start(sg, sgsin[:, :])
                    if sc == 3:
                        nc.sync.dma_start(wd_sb, wdG[:, :, :])

                def rope_vt(sc):
                    psums = qkv_ps[sc]
                    ssl = slice(sc * FD, (sc + 1) * FD)
                    for f in range(3):
                        dst = qr[:, f, ssl] if f < 2 else kr[:, ssl]
                        qt = rtmp.tile([P, FD], bf16, tag="qt", name=f"qt_{sc}_{f}")
                        nc.scalar.copy(qt, psums[f])
                        # rotate_half = two partition-shifted DVE copies
                        ts = rtmp.tile([P, FD], bf16, tag="ts", name=f"ts_{sc}_{f}")
                        nc.vector.tensor_copy(ts[0:64, :], qt[64:128, :])
                        nc.vector.tensor_copy(ts[64:128, :], qt[0:64, :])
                        tt = rtmp.tile([P, FD], bf16, tag="tt", name=f"tt_{sc}_{f}")
                        nc.vector.tensor_mul(tt, ts, sg[:, ssl])
                        nc.vector.tensor_mul(dst, qt, cq[:, ssl])
                        nc.vector.tensor_add(dst, dst, tt)
                    nc.scalar.copy(vT[:, ssl], psums[3])
                    for jj in range(4):
                        j = 4 * sc + jj
                        nc.sync.dma_start_transpose(vn[:, j, :], vT[:, j * P:(j + 1) * P])

                # chunk 0: plain
                for _ in qkv_units(0):
                    pass
                rope_vt(0)
                # chunk 1 + attn(0): 4 js at units 6,9,12,15; ctx 2 units later
                attn_begin(0)
                sched_s = {6: 0, 9: 1, 12: 2, 15: 3}
                sched_c = {8: 0, 11: 1, 14: 2}
                pend0 = {}
                for i, _ in enumerate(qkv_units(1)):
                    if i in sched_s:
                        pend0[sched_s[i]] = attn_step(0, sched_s[i], psA)
                    if i in sched_c:
                        attn_ctx(0, pend0.pop(sched_c[i]))
                attn_ctx(0, pend0.pop(3))
                attn_tail(0, psA)
                rope_vt(1)
                # chunks 2,3 + attn(1): 8 js over 32 units, spacing 4
                attn_begin(1)
                pend1 = {}
                pend2 = []
                base = 0
                for sc in (2, 3):
                    for i, _ in enumerate(qkv_units(sc)):
                        u = base + i
                        if u >= 3 and (u - 3) % 3 == 0 and (u - 3) // 3 < 8:
                            jx = (u - 3) // 3
                            pend1[jx] = attn_step(1, jx, psA)
                        if u >= 5 and (u - 5) % 3 == 0 and (u - 5) // 3 < 8:
                            attn_ctx(1, pend1.pop((u - 5) // 3))
                        if u == 27:
                            # attn(1) fully drained by u=26; start attn(2)'s
                            # producer side under the remaining QKV stream
                            attn_tail(1, psA)
                            attn_begin(2)
                            pend2.append(attn_step(2, 0, psA))
                        if u == 30:
                            pend2.append(attn_step(2, 1, psA))
                    if sc == 2:
                        rope_vt(2)
                    base += 16
                rope_vt(3)

            # ---- Phase B: attn(2/3) pipelined + dense ----
            def make_dense_units(pool):
                def dense_units(c, tail):
                    for st in range(4):
                        ot = outp.tile([P, 4, FD], bf16, tag="ot", name=f"ot_{c}_{st}")
                        for eo in range(4):
                            op = pool.tile([P, FD], f32, tag="o", name=f"o_{c}_{st}_{eo}")
                            for h in range(2):
                                nc.tensor.matmul(
                                    op,
                                    lhsT=all_csb[(c, h)][:, st * P:(st + 1) * P],
                                    rhs=wd_sb[:, h, eo * FD:(eo + 1) * FD],
                                    start=(h == 0), stop=(h == 1),
                                )
                            if (eo % 2) if tail else (eo == 3):
                                nc.scalar.copy(ot[:, eo, :], op)
                            else:
                                nc.vector.tensor_copy(ot[:, eo, :], op)
                            if tail and c == 3 and st == 3:
                                nc.sync.dma_start(out[c, st, :, eo, :], ot[:, eo, :])
                            elif eo == 3:
                                nc.sync.dma_start(out[c, st], ot)
                            yield
                return dense_units

            with tc.tile_pool(name="ps_sB", bufs=2, space="PSUM") as psB, \
                 tc.tile_pool(name="ps_o", bufs=2, space="PSUM") as ps_o:
                dense_units = make_dense_units(ps_o)

                def emit_attn_B(c, dq, pend=None, jstart=0):
                    nj = 4 * c + 4
                    if pend is None:
                        attn_begin(c)
                        pend = []
                    nd = 0
                    for j in range(jstart, nj):
                        pend.append(attn_step(c, j, psB))
                        if dq is not None:
                            want = (j + 1 - jstart) * 16 // (nj - jstart)
                            while nd < want:
                                next(dq)
                                nd += 1
                        if len(pend) >= 3:
                            attn_ctx(c, pend.pop(0))
                    while pend:
                        attn_ctx(c, pend.pop(0))
                    if dq is not None:
                        for _ in dq:
                            pass

                emit_attn_B(2, dense_units(0, False), pend=pend2, jstart=2)
                attn_tail(2, psB)
                emit_attn_B(3, dense_units(1, False))
            with tc.tile_pool(name="ps_d", bufs=4, space="PSUM") as ps_d:
                dense_tail = make_dense_units(ps_d)
                # tail(3)'s denominator matmuls wait on the DVE acc chains;
                # let dense(2) stream on the PE while those drain
                dq2 = dense_tail(2, True)
                for _ in range(10):
                    next(dq2)
                attn_tail(3, ps_d, ptag="o")
                for _ in dq2:
                    pass
                for _ in dense_tail(3, True):
                    pass
    nc.compile()
    return nc


def make_in_maps(x, w_qkv, w_dense):
    x = np.asarray(x, np.float32).reshape(S, E)
    w_qkv = np.asarray(w_qkv, np.float32)
    w_dense = np.asarray(w_dense, np.float32)
    # x^T tiled to [sc, g, p, j, f] so each 512KB DMA block is contiguous
    xT = np.ascontiguousarray(x.T)
    xG = np.ascontiguousarray(
        xT.reshape(NG, 4, P, NSC, FD).transpose(3, 0, 2, 1, 4)
    ).astype(BF)
    consts = _host_constants()
    in_maps = []
    scale = np.float64(1.0 / np.sqrt(D))
    for d in range(NCORES):
        g = d // 2
        wq = w_qkv[2 * d * P:(2 * d + 2) * P] * scale
        wk = w_qkv[H * D + g * P: H * D + (g + 1) * P]
        wv = w_qkv[H * D + KVH * D + g * P: H * D + KVH * D + (g + 1) * P]
        wqkvT_d = np.ascontiguousarray(np.concatenate([wq, wk, wv], 0).T)
        wG_d = np.ascontiguousarray(
            wqkvT_d.reshape(NG, 4, P, FLOC).transpose(0, 2, 1, 3)
        ).astype(BF)
        wdT_d = w_dense[:, 2 * d * P:(2 * d + 2) * P].T  # [2P, S]
        wdG_d = np.ascontiguousarray(
            wdT_d.reshape(2, P, S).transpose(1, 0, 2)
        ).astype(BF)
        m = {"xG": xG, "wG": wG_d, "wdG": wdG_d}
        m.update(consts)
        in_maps.append(m)
    return in_maps


def kernel(x, w_qkv, w_dense):
    global LAST_RESULT, _BASS_CACHE
    from concourse.bass_utils import run_bass_kernel_spmd

    in_maps = make_in_maps(x, w_qkv, w_dense)
    if _BASS_CACHE is None:
        _BASS_CACHE = _build_bass()
    res = run_bass_kernel_spmd(_BASS_CACHE, in_maps, core_ids=list(range(NCORES)))
    LAST_RESULT = res
    # sum partials over cores; [c, st, p, eo, f] flattens straight to [s, e]
    acc = np.zeros((NSC, 4, P, 4, FD), np.float32)
    for r in res.results:
        acc += r["out"].astype(np.float32)
    return np.ascontiguousarray(acc.reshape(S, E)).reshape(B, S, E)


# revision 32
# speedup vs baseline: 1.0128x; 1.0128x over previous
"""Tensor-parallel MultiHeadAttention (QKV + RoPE + GQA causal SDPA + dense)
for 8 Trainium2 NeuronCores — bf16, software-pipelined edition.

Sharding (TP as in TPMultiHeadAttention): core d owns query heads {2d, 2d+1}
and the single kv head d//2 (kv heads replicated across core pairs), plus the
matching 256 columns of the dense projection. Each core produces a full-shape
partial output; the all-reduce is a host-side sum over the 8 bf16 partials.

All matmul operands are bf16 (same PE column rate as float32r but half the
LDWEIGHTS time, half the DMA bytes, 2x DVE rate); PSUM accumulation is fp32.

Schedule highlights (from perfetto analysis of earlier revisions):
  - exp on ScalarE (616ns/tile) is slower than a score+ctx matmul pair
    (432ns), so the two heads' score tiles share one 2-bank PSUM tile and a
    single exp instruction covers both ([128, 2, n]).
  - attention for chunks 0 and 1 trickles INSIDE the QKV phase (the PE has
    54us of projection work to hide the exp latency); chunks 2 and 3 are
    software-pipelined with ctx lagging scores, with dense units of earlier
    chunks metered in as PE filler.
  - v^T -> v transposes use the DMA XBAR (16-bit only), freeing PSUM banks
    and the PE; rotate_half is two partition-shifted DVE copies.
  - softmax denominators: bf16 DVE accumulation chains, column-summed by a
    ones-vector matmul (borrowing a PSUM bank from the sd/o rings), DVE
    reciprocal, gpsimd partition broadcast.  The ctx PSUM banks are released
    early by an unscaled ScalarE copy; the 1/denominator multiply happens in
    SBUF off the ring-critical path.  gpsimd's ucode library load (~7us) is
    prefetched by a dummy broadcast at kernel start, and the rope/mask
    tables ride the otherwise-idle gpsimd DMA ring.
"""

import numpy as np
import ml_dtypes

B, S, E = 1, 2048, 2048
H, KVH, D = 16, 4, 128
NCORES = 8
P = 128
FD = 512            # matmul moving free dim == one fp32 PSUM bank
NE = E // P         # 16 contraction tiles over the embedding dim
NG = 4              # eo-groups of 4 (one 512KB DMA each)
NSC = S // FD       # 4 sequence chunks
NST = S // P        # 16 sequence tiles
FLOC = 4 * P        # local fused qkv rows per core (2 q heads + k + v)
ROPE_BASE = 10000.0
# causally visible query sub-range start for diagonal sk tile o
DIAG_START = (0, 128, 256, 384)
BF = ml_dtypes.bfloat16

LAST_RESULT = None
_BASS_CACHE = None


def _rope_tables():
    inv = 1.0 / (ROPE_BASE ** (np.arange(0, D, 2, dtype=np.float64) / D))
    t = np.arange(S, dtype=np.float64)
    freqs = np.outer(t, inv)
    emb = np.concatenate([freqs, freqs], axis=-1)  # [S, D]
    return np.cos(emb), np.sin(emb)


def _host_constants():
    cos, sin = _rope_tables()
    cos_ds = np.ascontiguousarray(cos.T)  # [D, S]
    sin_ds = np.ascontiguousarray(sin.T)
    # sign-folded sin for the partition-shifted rotate-half:
    # tt[d] = qt[(d+64)%128] * sg[d],  sg = [-sin[:64]; +sin[64:]]
    sg = np.concatenate([-sin_ds[:64], sin_ds[64:]], axis=0)
    r_idx = np.arange(P)[:, None]
    c_idx = np.arange(P)[None, :]
    tri = (r_idx <= c_idx).astype(np.float64)
    return {
        "cosr": cos_ds.astype(BF),
        "sgsin": sg.astype(BF),
        "trim": tri.astype(BF),
        "ones": np.ones((P, 1), np.float64).astype(BF),
    }


def _build_bass():
    import concourse.mybir as mybir
    import concourse.tile as tile
    from concourse import bacc

    f32 = mybir.dt.float32
    bf16 = mybir.dt.bfloat16
    Exp = mybir.ActivationFunctionType.Exp

    nc = bacc.Bacc(None, target_bir_lowering=False, name="mha_tp8_v3")
    xG = nc.dram_tensor("xG", [NSC, NG, P, 4, FD], bf16, kind="ExternalInput")
    wG = nc.dram_tensor("wG", [NG, P, 4, FLOC], bf16, kind="ExternalInput")
    wdG = nc.dram_tensor("wdG", [P, 2, S], bf16, kind="ExternalInput")
    cosr = nc.dram_tensor("cosr", [P, S], bf16, kind="ExternalInput")
    sgsin = nc.dram_tensor("sgsin", [P, S], bf16, kind="ExternalInput")
    trim = nc.dram_tensor("trim", [P, P], bf16, kind="ExternalInput")
    ones = nc.dram_tensor("ones", [P, 1], bf16, kind="ExternalInput")
    out = nc.dram_tensor("out", [NSC, 4, P, 4, FD], bf16, kind="ExternalOutput")

    with tile.TileContext(nc) as tc:
        with tc.tile_pool(name="const", bufs=1) as const, \
             tc.tile_pool(name="ps_ctx", bufs=2, space="PSUM") as ps_ctx, \
             tc.tile_pool(name="xs_p", bufs=5) as xpool, \
             tc.tile_pool(name="rtmp", bufs=3) as rtmp, \
             tc.tile_pool(name="pt_p", bufs=8) as ptp, \
             tc.tile_pool(name="acc_p", bufs=2) as accp, \
             tc.tile_pool(name="dn_p", bufs=2) as dnp, \
             tc.tile_pool(name="ctx_p", bufs=3) as ctxp, \
             tc.tile_pool(name="out_p", bufs=3) as outp:
            w_sb = const.tile([P, NE, FLOC], bf16, name="w_sb")
            cq = const.tile([P, S], bf16, name="cq")
            sg = const.tile([P, S], bf16, name="sg")
            mk = const.tile([P, P], bf16, name="mk")
            wd_sb = const.tile([P, 2, S], bf16, name="wd_sb")
            qr = const.tile([P, 2, S], bf16, name="qr")
            kr = const.tile([P, S], bf16, name="kr")
            vT = const.tile([P, S], bf16, name="vT")
            vn = const.tile([P, NST, P], bf16, name="vn")
            on = const.tile([P, 1], bf16, name="on")
            warm = const.tile([P, 8], bf16, name="warm")

            # ---- shared attention machinery (paired heads per j-tile) ----
            st_ = {}          # per-chunk attention state
            all_csb = {}

            def attn_begin(c):
                two = c >= 1
                ctxps = [
                    ps_ctx.tile([P, FD], f32, tag="ctx", name=f"ctx_{c}_{h}")
                    for h in range(2)
                ]
                accs = [
                    accp.tile([P, 2, FD], bf16, tag=f"acc{ch}", name=f"acc_{c}_{ch}")
                    for ch in range(2 if two else 1)
                ]
                st_[c] = (ctxps, accs, two)

            def attn_step(c, j, sdpool):
                ctxps, accs, two = st_[c]
                o = j - 4 * c
                so = DIAG_START[o] if o >= 0 else 0
                n = FD - so
                sd = sdpool.tile([P, 2, FD], f32, tag="sd", name=f"sd_{c}_{j}")
                for h in range(2):
                    nc.tensor.matmul(
                        sd[:, h, :n],
                        lhsT=kr[:, j * P:(j + 1) * P],
                        rhs=qr[:, h, c * FD + so:(c + 1) * FD],
                        start=True, stop=True,
                    )
                pt = ptp.tile([P, 2, FD], bf16, tag="pt", name=f"pt_{c}_{j}")
                nc.scalar.activation(pt[:, :, :n], sd[:, :, :n], Exp)
                if o >= 0:
                    for h in range(2):
                        nc.vector.tensor_mul(pt[:, h, :P], pt[:, h, :P], mk)
                acc = accs[j % 2 if two else 0]
                if j < (2 if two else 1):
                    nc.vector.tensor_copy(acc, pt)
                else:
                    nc.vector.tensor_add(acc[:, :, so:], acc[:, :, so:], pt[:, :, :n])
                return (j, pt, so, n)

            def attn_ctx(c, ent):
                ctxps, _, _ = st_[c]
                nj = 4 * c + 4
                j, pt, so, n = ent
                for h in range(2):
                    nc.tensor.matmul(
                        ctxps[h][:, so:],
                        lhsT=vn[:, j, :],
                        rhs=pt[:, h, :n],
                        start=(j == 0), stop=(j == nj - 1),
                    )

            def attn_tail(c, sdpool, ptag="sd"):
                ctxps, accs, two = st_[c]
                crs = []
                for h in range(2):
                    # unscaled PSUM->SBUF copy releases the ctx bank ~0.7us
                    # after the last ctx matmul (the scale chain is ~2.5us
                    # and would otherwise stall the next chunk's ctx ring)
                    cr = ctxp.tile([P, FD], bf16, tag=f"cr{h}", name=f"cr_{c}_{h}")
                    nc.scalar.copy(cr, ctxps[h])
                    crs.append(cr)
                for h in range(2):
                    # column sums via a ones-vector matmul into the sd ring
                    rp = sdpool.tile([1, FD], f32, tag=ptag, name=f"rp_{c}_{h}")
                    nc.tensor.matmul(rp, lhsT=on, rhs=accs[0][:, h, :],
                                     start=True, stop=not two)
                    if two:
                        nc.tensor.matmul(rp, lhsT=on, rhs=accs[1][:, h, :],
                                         start=False, stop=True)
                    rec = dnp.tile([1, FD], f32, tag=f"rec{h}", name=f"rec_{c}_{h}")
                    nc.vector.reciprocal_approx_fast(rec, rp)
                    rb = dnp.tile([P, FD], f32, tag=f"rb{h}", name=f"rb_{c}_{h}")
                    nc.gpsimd.partition_broadcast(rb, rec)
                    ct = ctxp.tile([P, FD], bf16, tag=f"ct{h}", name=f"csb_{c}_{h}")
                    nc.vector.tensor_mul(ct, crs[h], rb)
                    all_csb[(c, h)] = ct

            # ---- Phase A: QKV + RoPE + v-transpose, attn(0/1) trickled ----
            with tc.tile_pool(name="ps_qkv", bufs=1, space="PSUM") as pqkv, \
                 tc.tile_pool(name="ps_sA", bufs=1, space="PSUM") as psA:
                # tables ride the idle gpsimd ring so the sync ring's first
                # w/x transfers keep most of the DMA bandwidth
                nc.gpsimd.dma_start(mk, trim[:, :])
                nc.gpsimd.partition_broadcast(warm, mk[0:1, 0:8])
                nc.gpsimd.dma_start(cq, cosr[:, :])
                nc.gpsimd.dma_start(sg, sgsin[:, :])
                nc.gpsimd.dma_start(on, ones[:, :])

                qkv_ps = {}

                def qkv_units(sc):
                    """Yields after each (g, j) group of 4 matmuls (~864ns PE)."""
                    psums = [
                        pqkv.tile([P, FD], f32, tag=f"qkv{f}", name=f"ps_qkv{f}_{sc}")
                        for f in range(4)
                    ]
                    qkv_ps[sc] = psums
                    for g in range(NG):
                        fine = sc == 0 and g == 0
                        if sc == 0 and not fine:
                            nc.scalar.dma_start(w_sb[:, 4 * g:4 * g + 4, :], wG[g])
                        xs = xpool.tile([P, 4, FD], bf16, tag="xs", name=f"xs_{sc}_{g}")
                        if not fine:
                            nc.sync.dma_start(xs, xG[sc, g])
                        for j in range(4):
                            if fine:
                                # 128KB pieces, weights on the idle scalar
                                # ring, so the first matmuls aren't starved
                                nc.scalar.dma_start(w_sb[:, j, :], wG[0, :, j, :])
                                nc.sync.dma_start(xs[:, j, :], xG[0, 0, :, j, :])
                            eo = 4 * g + j
                            for f in range(4):
                                nc.tensor.matmul(
                                    psums[f],
                                    lhsT=w_sb[:, eo, f * P:(f + 1) * P],
                                    rhs=xs[:, j, :],
                                    start=(eo == 0),
                                    stop=(eo == NE - 1),
                                )
                            yield
                    if sc == 3:
                        nc.sync.dma_start(wd_sb, wdG[:, :, :])

                def rope_vt(sc):
                    psums = qkv_ps[sc]
                    ssl = slice(sc * FD, (sc + 1) * FD)
                    for f in range(3):
                        dst = qr[:, f, ssl] if f < 2 else kr[:, ssl]
                        qt = rtmp.tile([P, FD], bf16, tag="qt", name=f"qt_{sc}_{f}")
                        nc.scalar.copy(qt, psums[f])
                        # rotate_half = two partition-shifted DVE copies
                        ts = rtmp.tile([P, FD], bf16, tag="ts", name=f"ts_{sc}_{f}")
                        nc.vector.tensor_copy(ts[0:64, :], qt[64:128, :])
                        nc.vector.tensor_copy(ts[64:128, :], qt[0:64, :])
                        tt = rtmp.tile([P, FD], bf16, tag="tt", name=f"tt_{sc}_{f}")
                        nc.vector.tensor_mul(tt, ts, sg[:, ssl])
                        nc.vector.tensor_mul(dst, qt, cq[:, ssl])
                        nc.vector.tensor_add(dst, dst, tt)
                    nc.scalar.copy(vT[:, ssl], psums[3])
                    for jj in range(4):
                        j = 4 * sc + jj
                        nc.sync.dma_start_transpose(vn[:, j, :], vT[:, j * P:(j + 1) * P])

                # chunk 0: plain
                for _ in qkv_units(0):
                    pass
                rope_vt(0)
                # chunk 1 + attn(0): 4 js at units 6,9,12,15; ctx 2 units later
                attn_begin(0)
                sched_s = {6: 0, 9: 1, 12: 2, 15: 3}
                sched_c = {8: 0, 11: 1, 14: 2}
                pend0 = {}
                for i, _ in enumerate(qkv_units(1)):
                    if i in sched_s:
                        pend0[sched_s[i]] = attn_step(0, sched_s[i], psA)
                    if i in sched_c:
                        attn_ctx(0, pend0.pop(sched_c[i]))
                attn_ctx(0, pend0.pop(3))
                attn_tail(0, psA)
                rope_vt(1)
                # chunks 2,3 + attn(1): 8 js over 32 units, spacing 4
                attn_begin(1)
                pend1 = {}
                pend2 = []
                base = 0
                for sc in (2, 3):
                    for i, _ in enumerate(qkv_units(sc)):
                        u = base + i
                        if u >= 3 and (u - 3) % 3 == 0 and (u - 3) // 3 < 8:
                            jx = (u - 3) // 3
                            pend1[jx] = attn_step(1, jx, psA)
                        if u >= 5 and (u - 5) % 3 == 0 and (u - 5) // 3 < 8:
                            attn_ctx(1, pend1.pop((u - 5) // 3))
                        if u == 27:
                            # attn(1) fully drained by u=26; start attn(2)'s
                            # producer side under the remaining QKV stream
                            attn_tail(1, psA)
                            attn_begin(2)
                            pend2.append(attn_step(2, 0, psA))
                        if u == 30:
                            pend2.append(attn_step(2, 1, psA))
                    if sc == 2:
                        rope_vt(2)
                    base += 16
                rope_vt(3)

            # ---- Phase B: attn(2/3) pipelined + dense ----
            def make_dense_units(pool):
                def dense_units(c, tail):
                    for st in range(4):
                        ot = outp.tile([P, 4, FD], bf16, tag="ot", name=f"ot_{c}_{st}")
                        for eo in range(4):
                            op = pool.tile([P, FD], f32, tag="o", name=f"o_{c}_{st}_{eo}")
                            for h in range(2):
                                nc.tensor.matmul(
                                    op,
                                    lhsT=all_csb[(c, h)][:, st * P:(st + 1) * P],
                                    rhs=wd_sb[:, h, eo * FD:(eo + 1) * FD],
                                    start=(h == 0), stop=(h == 1),
                                )
                            if (eo % 2) if tail else (eo == 3):
                                nc.scalar.copy(ot[:, eo, :], op)
                            else:
                                nc.vector.tensor_copy(ot[:, eo, :], op)
                            if tail and c == 3 and st == 3:
                                nc.sync.dma_start(out[c, st, :, eo, :], ot[:, eo, :])
                            elif eo == 3:
                                nc.sync.dma_start(out[c, st], ot)
                            yield
                return dense_units

            with tc.tile_pool(name="ps_sB", bufs=2, space="PSUM") as psB, \
                 tc.tile_pool(name="ps_o", bufs=2, space="PSUM") as ps_o:
                dense_units = make_dense_units(ps_o)

                def emit_attn_B(c, dq, pend=None, jstart=0):
                    nj = 4 * c + 4
                    if pend is None:
                        attn_begin(c)
                        pend = []
                    nd = 0
                    for j in range(jstart, nj):
                        pend.append(attn_step(c, j, psB))
                        if dq is not None:
                            want = (j + 1 - jstart) * 16 // (nj - jstart)
                            while nd < want:
                                next(dq)
                                nd += 1
                        if len(pend) >= 3:
                            attn_ctx(c, pend.pop(0))
                    while pend:
                        attn_ctx(c, pend.pop(0))
                    if dq is not None:
                        for _ in dq:
                            pass

                emit_attn_B(2, dense_units(0, False), pend=pend2, jstart=2)
                attn_tail(2, psB)
                emit_attn_B(3, dense_units(1, False))
            with tc.tile_pool(name="ps_d", bufs=4, space="PSUM") as ps_d:
                dense_tail = make_dense_units(ps_d)
                # tail(3)'s denominator matmuls wait on the DVE acc chains;
                # let dense(2) stream on the PE while those drain
                dq2 = dense_tail(2, True)
                for _ in range(10):
                    next(dq2)
                attn_tail(3, ps_d, ptag="o")
                for _ in dq2:
                    pass
                for _ in dense_tail(3, True):
                    pass
    nc.compile()
    return nc


def make_in_maps(x, w_qkv, w_dense):
    x = np.asarray(x, np.float32).reshape(S, E)
    w_qkv = np.asarray(w_qkv, np.float32)
    w_dense = np.asarray(w_dense, np.float32)
    # x^T tiled to [sc, g, p, j, f] so each 512KB DMA block is contiguous
    xT = np.ascontiguousarray(x.T)
    xG = np.ascontiguousarray(
        xT.reshape(NG, 4, P, NSC, FD).transpose(3, 0, 2, 1, 4)
    ).astype(BF)
    consts = _host_constants()
    in_maps = []
    scale = np.float64(1.0 / np.sqrt(D))
    for d in range(NCORES):
        g = d // 2
        wq = w_qkv[2 * d * P:(2 * d + 2) * P] * scale
        wk = w_qkv[H * D + g * P: H * D + (g + 1) * P]
        wv = w_qkv[H * D + KVH * D + g * P: H * D + KVH * D + (g + 1) * P]
        wqkvT_d = np.ascontiguousarray(np.concatenate([wq, wk, wv], 0).T)
        wG_d = np.ascontiguousarray(
            wqkvT_d.reshape(NG, 4, P, FLOC).transpose(0, 2, 1, 3)
        ).astype(BF)
        wdT_d = w_dense[:, 2 * d * P:(2 * d + 2) * P].T  # [2P, S]
        wdG_d = np.ascontiguousarray(
            wdT_d.reshape(2, P, S).transpose(1, 0, 2)
        ).astype(BF)
        m = {"xG": xG, "wG": wG_d, "wdG": wdG_d}
        m.update(consts)
        in_maps.append(m)
    return in_maps


def kernel(x, w_qkv, w_dense):
    global LAST_RESULT, _BASS_CACHE
    from concourse.bass_utils import run_bass_kernel_spmd

    in_maps = make_in_maps(x, w_qkv, w_dense)
    if _BASS_CACHE is None:
        _BASS_CACHE = _build_bass()
    res = run_bass_kernel_spmd(_BASS_CACHE, in_maps, core_ids=list(range(NCORES)))
    LAST_RESULT = res
    # sum partials over cores; [c, st, p, eo, f] flattens straight to [s, e]
    acc = np.zeros((NSC, 4, P, 4, FD), np.float32)
    for r in res.results:
        acc += r["out"].astype(np.float32)
    return np.ascontiguousarray(acc.reshape(S, E)).reshape(B, S, E)
